# revision 1
# baseline (speedup 1.0000x reference)
"""Masked 5x5 conv (PixelCNN 'A' mask) on 8 Trainium2 NeuronCores.

Problem (hardcoded): x[4,192,128,128] f32, weight[384,192,5,5] f32,
bias[384] f32, mask[4,1,128,128] i32.
out = where(window_any(mask), conv(x, weight*maskA) + bias, 0).

The 'A' causal mask keeps 12 of 25 taps: rows kh=0,1 fully, row kh=2 only
kw=0,1 -- i.e. every tap reads the current output row or rows above it.

Sharding: core c = (batch b = c//2, row-half = c%2). Each core computes one
batch's 64 output rows for all 384 out channels (3 M=128 chunks).

Per output tile [128 cout, 4 rows x 128 cols = 512] we accumulate 18 K=128
bf16 matmuls into one PSUM bank:
  - 12 taps x channel-chunk ci[0:128]  (from tile xa)
  - 5 tap-PAIRS x ci[128:192]          (from tile xb: lower 64 partitions =
    ci[128:192] data, upper 64 = same data shifted 1 col, so one K=128
    matmul covers two taps that differ by (0,+1))
  - 1 tap-pair (0,4)+(1,4) x ci[128:192] (tile xc: upper shifted one row)
Epilogue: one DVE scalar_tensor_tensor: out = (psum + bias) * valid.
"""

import numpy as np
import ml_dtypes

import concourse.bass as bass
import concourse.tile as tile
from concourse import mybir
from concourse.bass_utils import run_bass_kernel_spmd

B, CIN, COUT, H, W = 4, 192, 384, 128, 128
KH = KW = 5
PAD = 2
NCORES = 8
HHALF = 64          # output rows per core
NROWS = HHALF + 2   # input rows staged per core (2 above)
WP = W + 4          # padded width
FLAT = NROWS * WP   # 66*132 = 8712
RB = 4              # output rows per block
NBLK = HHALF // RB  # 16 blocks
NFREE = RB * W      # 512 = one PSUM bank of fp32

# Active taps of the 'A' mask, (kh, kw)
TAPS = [(0, 0), (0, 1), (0, 2), (0, 3), (0, 4),
        (1, 0), (1, 1), (1, 2), (1, 3), (1, 4),
        (2, 0), (2, 1)]
# ci[128:192] handled as pairs packed into K=128 matmuls.
# slab xb (upper shifted +1 element = +1 col): pairs differing by (0,1)
PAIRS_XB = [((0, 0), (0, 1)), ((0, 2), (0, 3)),
            ((1, 0), (1, 1)), ((1, 2), (1, 3)), ((2, 0), (2, 1))]
# slab xc (upper shifted +132 elements = +1 row): the leftover pair
PAIR_XC = ((0, 4), (1, 4))

BF16 = ml_dtypes.bfloat16


def _build_program():
    """Raw Bass (no Tile): this walrus build rejects instructions carrying
    more than ~1 embedded sync wait, so all synchronization is standalone
    wait_ge instructions with manually-managed semaphores.

    Schedule (per core, ~210us):
      - PE pre-warm: 14 dummy matmuls during the initial DMA wait flip the
        HAM clock gate to 2.4 GHz before the real stream begins.
      - Input DMAs stream in prioritized serialized waves (queues are
        ~45-90 GB/s each, ~358 GB/s aggregate).
      - Phase A runs the 12 xa-slots of tiles 0..7 as soon as the first
        weight/xa chunks land; phase B completes those tiles with the
        xb/xc pair slots once those tensors arrive; then steady state:
        18 K=128 matmuls per [128 cout x 512 spatial] PSUM tile.
      - DVE fuses (psum + bias) * valid into one scalar_tensor_tensor per
        tile, writing a bf16 staging buffer; outputs stream out in 2-tile
        chunks with a tapered, 2-way-split final chunk."""
    nc = bass.Bass()
    bf = mybir.dt.bfloat16
    f32 = mybir.dt.float32

    xa_d = nc.dram_tensor("xa", [128, FLAT], bf, kind="ExternalInput")
    xb_d = nc.dram_tensor("xb", [128, FLAT], bf, kind="ExternalInput")
    xc_d = nc.dram_tensor("xc", [128, FLAT], bf, kind="ExternalInput")
    wt_d = nc.dram_tensor("wt", [128, 18 * COUT], bf, kind="ExternalInput")
    bt_d = nc.dram_tensor("bt", [128, 3], f32, kind="ExternalInput")
    vt_d = nc.dram_tensor("vt", [128, HHALF * W], bf, kind="ExternalInput")
    out_d = nc.dram_tensor("out", [128, 3 * HHALF * W], bf, kind="ExternalOutput")

    NPS = 8           # psum banks in rotation
    PHA = 8           # tiles 0..PHA-1 run split-phase (xa first, xb/xc later)
    XA1 = 38 * WP     # xa chunk 1 covers input rows 0..37 (output blocks 0..7)
    OCH = 2           # out-DMA granularity: blocks per chunk
    NT = 3 * NBLK     # 48 tiles

    from contextlib import ExitStack
    with ExitStack() as ctx:
        xa_t = ctx.enter_context(nc.sbuf_tensor([128, FLAT], bf))
        xb_t = ctx.enter_context(nc.sbuf_tensor([128, FLAT], bf))
        xc_t = ctx.enter_context(nc.sbuf_tensor([128, FLAT], bf))
        wt_t = ctx.enter_context(nc.sbuf_tensor([128, 18 * COUT], bf))
        bt_t = ctx.enter_context(nc.sbuf_tensor([128, 3], f32))
        vt_t = ctx.enter_context(nc.sbuf_tensor([128, HHALF * W], bf))
        st_t = ctx.enter_context(nc.sbuf_tensor([128, 3 * HHALF * W], bf))
        ps_t = ctx.enter_context(nc.psum_tensor([128, NPS * NFREE], f32))
        da0 = ctx.enter_context(nc.semaphore("da0"))
        da1 = ctx.enter_context(nc.semaphore("da1"))
        da2 = ctx.enter_context(nc.semaphore("da2"))
        db1 = ctx.enter_context(nc.semaphore("db1"))
        db2 = ctx.enter_context(nc.semaphore("db2"))
        dc1 = ctx.enter_context(nc.semaphore("dc1"))
        dc2 = ctx.enter_context(nc.semaphore("dc2"))
        dwt1 = ctx.enter_context(nc.semaphore("dwt1"))
        dwt2 = ctx.enter_context(nc.semaphore("dwt2"))
        drest = ctx.enter_context(nc.semaphore("drest"))
        pes = ctx.enter_context(nc.semaphore("pes"))
        dve = ctx.enter_context(nc.semaphore("dve"))
        dout = ctx.enter_context(nc.semaphore("dout"))
        warm = ctx.enter_context(nc.semaphore("warm"))
        block = ctx.enter_context(nc.Block())
        xa_v = xa_t[:].rearrange("p (r c) -> p r c", c=WP)
        xb_v = xb_t[:].rearrange("p (r c) -> p r c", c=WP)
        xc_v = xc_t[:].rearrange("p (r c) -> p r c", c=WP)

        # (global weight-slot index, view, kh, kw)
        slots_a = [(s, xa_v, kh, kw) for s, (kh, kw) in enumerate(TAPS)]
        slots_bc = [(12 + i, xb_v, ta[0], ta[1])
                    for i, (ta, _tb) in enumerate(PAIRS_XB)]
        slots_bc += [(17, xc_v, PAIR_XC[0][0], PAIR_XC[0][1])]

        def emit_mms(tensor, k, sl, start, stop):
            m, blk = divmod(k, NBLK)
            j0 = blk * RB
            ps = ps_t[:, (k % NPS) * NFREE:(k % NPS + 1) * NFREE]
            n = len(sl)
            for i, (s, view, kh, kw) in enumerate(sl):
                mm = nc.tensor.matmul(
                    ps,
                    wt_t[:, s * COUT + m * 128: s * COUT + (m + 1) * 128],
                    view[:, j0 + kh: j0 + kh + RB, kw: kw + W],
                    start=(start and i == 0),
                    stop=(stop and i == n - 1),
                )
                if stop and i == n - 1:
                    mm.then_inc(pes, 1)

        @block.sync
        def _(sync):
            # DMA queues give ~45-90 GB/s per stream and ~358 GB/s aggregate,
            # so stream in prioritized serialized waves, each wave split
            # across a few queues. Wave 1a covers the very first matmuls.
            WT1 = 12 * COUT   # wt cols for the 12 xa slots
            XA0 = 14 * WP     # xa rows 0..13: blocks 0..2
            def split2(dst, src, lo, hi, sem):
                mid = ((lo + hi) // 2 // 4) * 4
                sync.dma_start(dst[:, lo:mid], src[:, lo:mid]).then_inc(sem, 16)
                sync.dma_start(dst[:, mid:hi], src[:, mid:hi]).then_inc(sem, 16)

            # wt1 in three chunks: per-queue BW (~90 GB/s) makes the largest
            # chunk the wave-1a critical path
            W3 = WT1 // 3 // 4 * 4
            sync.dma_start(wt_t[:, 0:W3], wt_d[:, 0:W3]).then_inc(dwt1, 16)
            sync.dma_start(wt_t[:, W3:2 * W3], wt_d[:, W3:2 * W3]).then_inc(dwt1, 16)
            sync.dma_start(wt_t[:, 2 * W3:WT1], wt_d[:, 2 * W3:WT1]).then_inc(dwt1, 16)
            split2(xa_t, xa_d, 0, XA0, da0)
            sync.wait_ge(dwt1, 48)
            sync.wait_ge(da0, 32)
            split2(xa_t, xa_d, XA0, XA1, da1)
            sync.wait_ge(da1, 32)
            # wave 2: phase-B inputs + DVE epilogue inputs; xb first
            split2(xb_t, xb_d, 0, XA1, db1)
            split2(xc_t, xc_d, 0, XA1, dc1)
            sync.dma_start(wt_t[:, WT1:], wt_d[:, WT1:]).then_inc(dwt2, 16)
            sync.dma_start(bt_t[:], bt_d[:]).then_inc(drest, 16)
            split2(vt_t, vt_d, 0, HHALF * W, drest)
            sync.wait_ge(db1, 32)
            sync.wait_ge(dc1, 32)
            # wave 3: steady-state remainders
            split2(xa_t, xa_d, XA1, FLAT, da2)
            split2(xb_t, xb_d, XA1, FLAT, db2)
            split2(xc_t, xc_d, XA1, FLAT, dc2)
            # output chunks of OCH tiles; the last two tiles go out singly
            # (the final DMA is the only one on the critical path, so the
            # smaller and more parallel it is, the shorter the tail)
            nch = NT // OCH
            ninc = 0
            for c in range(nch):
                lo, hi = c * OCH * NFREE, (c + 1) * OCH * NFREE
                if c == nch - 1:
                    # tile 46, then the two halves of the split tile 47
                    sync.wait_ge(dve, NT - 1)
                    mid = lo + NFREE
                    sync.dma_start(out_d[:, lo:mid], st_t[:, lo:mid]).then_inc(dout, 16)
                    mid2 = mid + NFREE // 2
                    sync.wait_ge(dve, NT)
                    sync.dma_start(out_d[:, mid:mid2], st_t[:, mid:mid2]).then_inc(dout, 16)
                    sync.wait_ge(dve, NT + 1)
                    split2(out_d, st_t, mid2, hi, dout)
                    ninc += 4
                else:
                    sync.wait_ge(dve, OCH * (c + 1))
                    sync.dma_start(out_d[:, lo:hi], st_t[:, lo:hi]).then_inc(dout, 16)
                    ninc += 1
            sync.wait_ge(dout, 16 * ninc)

        @block.tensor
        def _(tensor):
            # pre-warm the PE HAM clock gate during the initial DMA wait:
            # ~5us of dummy matmuls (zeros into bank 7, which tile 7
            # later clears with start=True) flips the PE to full clock
            # before the real stream begins. st_t is idle SBUF.
            # 12 dummies x ~426ns cold = ~5us: ends about when wave-1 DMA
            # lands, and >3.4us of PE activity flips the clock to 2.4GHz
            tensor.wait_ge(warm, 1)
            for _ in range(11):
                nc.tensor.matmul(
                    ps_t[:, 7 * NFREE:8 * NFREE],
                    st_t[0:1, 0:128],
                    st_t[0:1, 0:NFREE],
                    start=True,
                    stop=True,
                )
            # phase A: xa-only accumulation for tiles 0..PHA-1, gated on the
            # just-in-time xa row chunks
            tensor.wait_ge(dwt1, 48)
            tensor.wait_ge(da0, 32)
            for k in range(3):
                emit_mms(tensor, k, slots_a, start=True, stop=False)
            tensor.wait_ge(da1, 32)
            for k in range(3, PHA):
                emit_mms(tensor, k, slots_a, start=True, stop=False)
            # phase B: finish tiles 0..PHA-1 with the xb/xc pair slots
            tensor.wait_ge(dwt2, 16)
            tensor.wait_ge(db1, 32)
            tensor.wait_ge(dc1, 32)
            for k in range(PHA):
                emit_mms(tensor, k, slots_bc, start=False, stop=True)
            # steady state
            tensor.wait_ge(da2, 32)
            tensor.wait_ge(db2, 32)
            tensor.wait_ge(dc2, 32)
            # one bank-reuse wait covers 4 tiles: tiles k..k+3 need at most
            # dve >= k+3-(NPS-1) = k-4, and DVE lags PE by well under the
            # 3-tile slack this leaves. Fewer waits = fewer PE queue stalls.
            for k in range(PHA, NT - 1):
                if (k - PHA) % 4 == 0:
                    tensor.wait_ge(dve, min(k + 3, NT - 1) - NPS + 1)
                emit_mms(tensor, k, slots_a, start=True, stop=False)
                emit_mms(tensor, k, slots_bc, start=False, stop=True)
            # final tile split into two 2-row groups (N=256 in half banks):
            # the first half's epilogue+DMA overlaps the second half's
            # matmuls, shortening the kernel tail
            k = NT - 1
            m, blk = divmod(k, NBLK)
            j0 = blk * RB
            for h in range(2):
                # halves in DIFFERENT banks (7, then 6): DVE reads half 1
                # while PE accumulates half 2, and same-bank PE-write +
                # DVE-read is a fatal PSUM collision. Bank 6 (tile 46) is
                # free once dve >= NT-1.
                if h == 1:
                    tensor.wait_ge(dve, NT - 1)
                ps_h = ps_t[:, (7 - h) * NFREE:(7 - h) * NFREE + NFREE // 2]
                for sl, is_last in ((slots_a, False), (slots_bc, True)):
                    n = len(sl)
                    for i, (s, view, kh, kw) in enumerate(sl):
                        mm = nc.tensor.matmul(
                            ps_h,
                            wt_t[:, s * COUT + m * 128: s * COUT + (m + 1) * 128],
                            view[:, j0 + 2 * h + kh: j0 + 2 * h + kh + RB // 2,
                                 kw: kw + W],
                            start=(sl is slots_a and i == 0),
                            stop=(is_last and i == n - 1),
                        )
                        if is_last and i == n - 1:
                            mm.then_inc(pes, 1)

        @block.vector
        def _(vector):
            nc.vector.memset(st_t[0:1, 0:NFREE], 0.0).then_inc(warm, 1)
            vector.wait_ge(drest, 48)  # bias + valid resident (3 chunks)
            for k in range(NT - 1):
                m, blk = divmod(k, NBLK)
                ps = ps_t[:, (k % NPS) * NFREE:(k % NPS + 1) * NFREE]
                vector.wait_ge(pes, k + 1)
                nc.vector.scalar_tensor_tensor(
                    st_t[:, k * NFREE:(k + 1) * NFREE],
                    ps,
                    bt_t[:, m:m + 1],
                    vt_t[:, blk * NFREE:(blk + 1) * NFREE],
                    mybir.AluOpType.add,
                    mybir.AluOpType.mult,
                ).then_inc(dve, 1)
            # final tile: two half-width epilogues matching the split groups
            k = NT - 1
            m, blk = divmod(k, NBLK)
            HF = NFREE // 2
            for h in range(2):
                ps_h = ps_t[:, (7 - h) * NFREE:(7 - h) * NFREE + HF]
                vector.wait_ge(pes, k + 1 + h)
                nc.vector.scalar_tensor_tensor(
                    st_t[:, k * NFREE + h * HF:k * NFREE + (h + 1) * HF],
                    ps_h,
                    bt_t[:, m:m + 1],
                    vt_t[:, blk * NFREE + h * HF:blk * NFREE + (h + 1) * HF],
                    mybir.AluOpType.add,
                    mybir.AluOpType.mult,
                ).then_inc(dve, 1)
    return nc


def _causal_mask():
    m = np.ones((KH, KW), dtype=np.float32)
    m[KH // 2, KW // 2:] = 0.0
    m[KH // 2 + 1:, :] = 0.0
    return m


def _prepare_in_maps(x, weight, bias, mask):
    # window-any of mask -> valid [B, H, W] float32
    ind = (np.asarray(mask)[:, 0] != 0)
    indp = np.zeros((B, H + 2 * PAD, W + 2 * PAD), dtype=bool)
    indp[:, PAD:PAD + H, PAD:PAD + W] = ind
    valid = np.zeros((B, H, W), dtype=bool)
    for dh in range(KH):
        for dw in range(KW):
            valid |= indp[:, dh:dh + H, dw:dw + W]
    valid_f = valid.astype(np.float32)

    w_bf = (np.asarray(weight, dtype=np.float32) * _causal_mask()[None, None]).astype(BF16)

    # 18 weight tiles [K=128, M=384] -> one SBUF image [128, 18, 384]
    wt = np.zeros((18, 128, COUT), dtype=BF16)
    for s, (kh, kw) in enumerate(TAPS):
        wt[s] = w_bf[:, 0:128, kh, kw].T
    for i, (ta, tb) in enumerate(PAIRS_XB):
        wt[12 + i, 0:64] = w_bf[:, 128:192, ta[0], ta[1]].T
        wt[12 + i, 64:128] = w_bf[:, 128:192, tb[0], tb[1]].T
    ta, tb = PAIR_XC
    wt[17, 0:64] = w_bf[:, 128:192, ta[0], ta[1]].T
    wt[17, 64:128] = w_bf[:, 128:192, tb[0], tb[1]].T
    wt_sb = np.ascontiguousarray(wt.transpose(1, 0, 2))

    bias_t = np.ascontiguousarray(
        np.asarray(bias, dtype=np.float32).reshape(3, 128).T)

    x_bf = np.asarray(x, dtype=np.float32).astype(BF16)

    in_maps = []
    for c in range(NCORES):
        b, half = c // 2, c % 2
        r0 = half * HHALF
        xp = np.zeros((CIN, NROWS, WP), dtype=BF16)
        lo = r0 - PAD
        src_lo = max(lo, 0)
        xp[:, src_lo - lo:, PAD:PAD + W] = x_bf[b, :, src_lo:r0 + HHALF, :]
        xf = xp.reshape(CIN, FLAT)
        x2 = xf[128:192]
        sh1 = np.zeros_like(x2)
        sh1[:, :-1] = x2[:, 1:]
        shr = np.zeros_like(x2)
        shr[:, :-WP] = x2[:, WP:]
        vrow = valid_f[b, r0:r0 + HHALF].reshape(1, HHALF * W).astype(BF16)
        vt = np.ascontiguousarray(np.broadcast_to(vrow, (128, HHALF * W)))
        in_maps.append({
            "xa": np.ascontiguousarray(xf[0:128]),
            "xb": np.ascontiguousarray(np.concatenate([x2, sh1], axis=0)),
            "xc": np.ascontiguousarray(np.concatenate([x2, shr], axis=0)),
            "wt": wt_sb.reshape(128, 18 * COUT),
            "bt": bias_t,
            "vt": vt,
        })
    return in_maps


def _assemble(results):
    out_full = np.zeros((B, COUT, H, W), dtype=np.float32)
    for c in range(NCORES):
        b, half = c // 2, c % 2
        o = np.asarray(results[c]["out"]).astype(np.float32)
        o4 = o.reshape(128, 3, HHALF, W).transpose(1, 0, 2, 3).reshape(COUT, HHALF, W)
        out_full[b, :, half * HHALF:(half + 1) * HHALF, :] = o4
    return out_full


def kernel(x, weight, bias, mask, _trace=False):
    in_maps = _prepare_in_maps(x, weight, bias, mask)
    nc = _build_program()
    res = run_bass_kernel_spmd(nc, in_maps, core_ids=list(range(NCORES)),
                               trace=_trace)
    out = _assemble(res.results)
    if _trace:
        return out, res
    return out



# revision 2
# speedup vs baseline: 1.1078x; 1.1078x over previous
"""Masked 5x5 conv (PixelCNN 'A' mask) on 8 Trainium2 NeuronCores.

Problem (hardcoded): x[4,192,128,128] f32, weight[384,192,5,5] f32,
bias[384] f32, mask[4,1,128,128] i32.
out = where(window_any(mask), conv(x, weight*maskA) + bias, 0).

The 'A' causal mask keeps 12 of 25 taps: rows kh=0,1 fully, row kh=2 only
kw=0,1 -- i.e. every tap reads the current output row or rows above it.

Sharding: core c = (batch b = c//2, row-half = c%2). Each core computes one
batch's 64 output rows for all 384 out channels (3 M=128 chunks).

Per output tile [128 cout, 4 rows x 128 cols = 512] we accumulate 16
matmuls into one PSUM bank:
  - 2 fp8 DoubleRow matmuls (K=256): tap pairs (0,0)+(1,0) and (0,2)+(1,2)
    on ci[0:128], operands e4m3 (x/8 and 8*w so products keep natural
    scale).  DoubleRow streams the two K-groups as dim-1 of a [128,2,...]
    AP; group 1 reads a row-shifted fp8 copy so the pair step is a fixed
    whole-tensor offset (16-byte aligned).
  - 8 bf16 K=128 matmuls for the remaining ci[0:128] taps (from tile xa)
  - 5 bf16 tap-PAIRS x ci[128:192]      (tile xb: lower 64 partitions =
    ci[128:192] data, upper 64 = same data shifted 1 col, so one K=128
    matmul covers two taps that differ by (0,+1))
  - 1 bf16 tap-pair (0,4)+(1,4) x ci[128:192] (tile xc: upper shifted +1 row)
4 of 18 K-slabs in fp8 keeps rel err ~0.017 (<2e-2) while cutting the PE
stream ~10%.
Epilogue: one DVE scalar_tensor_tensor: out = (psum + bias) * valid.
"""

import numpy as np
import ml_dtypes

import concourse.bass as bass
import concourse.tile as tile
from concourse import mybir
from concourse.bass_utils import run_bass_kernel_spmd

B, CIN, COUT, H, W = 4, 192, 384, 128, 128
KH = KW = 5
PAD = 2
NCORES = 8
HHALF = 64          # output rows per core
NROWS = HHALF + 2   # input rows staged per core (2 above)
WP = W + 4          # padded width (bf16 tensors)
FLAT = NROWS * WP   # 66*132 = 8712
WP8 = 144           # fp8 row pitch (16B-aligned so DoubleRow step%16==0)
FLAT8 = NROWS * WP8  # 66*144 = 9504
RB = 4              # output rows per block
NBLK = HHALF // RB  # 16 blocks
NFREE = RB * W      # 512 = one PSUM bank of fp32
NT = 3 * NBLK       # 48 tiles

# bf16 xa taps (ci 0:128) -- the 8 'A'-mask taps not covered by fp8 pairs
TAPS_BF = [(0, 1), (0, 3), (0, 4),
           (1, 1), (1, 3), (1, 4),
           (2, 0), (2, 1)]
# fp8 DoubleRow pairs: taps (0,kw)+(1,kw) on ci[0:128]
DR_KW = [0, 2]
# ci[128:192] handled as bf16 pairs packed into K=128 matmuls.
# slab xb (upper shifted +1 element = +1 col): pairs differing by (0,1)
PAIRS_XB = [((0, 0), (0, 1)), ((0, 2), (0, 3)),
            ((1, 0), (1, 1)), ((1, 2), (1, 3)), ((2, 0), (2, 1))]
# slab xc (upper shifted +132 elements = +1 row): the leftover pair
PAIR_XC = ((0, 4), (1, 4))

NSLOT = len(TAPS_BF) + len(PAIRS_XB) + 1   # 14 bf16 weight slots
SBC0 = len(TAPS_BF)                        # first xb/xc slot index

BF16 = ml_dtypes.bfloat16
F8 = ml_dtypes.float8_e4m3


def _build_program():
    """Raw Bass (no Tile): this walrus build rejects instructions carrying
    more than ~1 embedded sync wait, so all synchronization is standalone
    wait_ge instructions with manually-managed semaphores.

    Schedule (per core):
      - PE pre-warm: dummy matmuls during the initial DMA wait flip the
        HAM clock gate toward 2.4 GHz before the real stream begins.
      - Input DMAs stream in prioritized FIFO waves (queues are ~45-90
        GB/s each, ~358 GB/s aggregate); wave-1a is kept tiny (m=0
        weights for the first slots + first rows of xa/x8) so real
        matmuls start ~10us in.
      - Phase A runs the 2 DR + 8 xa slots of tiles 0..7 as the first
        x rows land; phase B completes those tiles with the xb/xc pair
        slots; then steady state: 16 matmuls per [128 x 512] PSUM tile.
      - DVE fuses (psum + bias) * valid into one scalar_tensor_tensor per
        tile, writing a bf16 staging buffer; outputs stream out in 2-tile
        chunks with a tapered, split final chunk."""
    nc = bass.Bass()
    bf = mybir.dt.bfloat16
    f8 = mybir.dt.float8e4
    f32 = mybir.dt.float32

    xa_d = nc.dram_tensor("xa", [128, FLAT], bf, kind="ExternalInput")
    x8_d = nc.dram_tensor("x8", [128, 2 * FLAT8], f8, kind="ExternalInput")
    xb_d = nc.dram_tensor("xb", [128, FLAT], bf, kind="ExternalInput")
    xc_d = nc.dram_tensor("xc", [128, FLAT], bf, kind="ExternalInput")
    wt_d = nc.dram_tensor("wt", [128, 3 * NSLOT * 128], bf, kind="ExternalInput")
    wdr_d = nc.dram_tensor("wdr", [128, 3 * 2 * 256], f8, kind="ExternalInput")
    bt_d = nc.dram_tensor("bt", [128, 3], f32, kind="ExternalInput")
    vt_d = nc.dram_tensor("vt", [128, HHALF * W], bf, kind="ExternalInput")
    out_d = nc.dram_tensor("out", [128, 3 * HHALF * W], bf, kind="ExternalOutput")

    NPS = 8           # psum banks in rotation
    PHA = 8           # tiles 0..PHA-1 run split-phase (xa/DR first, xb/xc later)
    OCH = 2           # out-DMA granularity: blocks per chunk
    NDUMMY = 7        # PE pre-warm matmuls
    DR = mybir.MatmulPerfMode.DoubleRow

    # row boundaries for the input waves (staged row index)
    R1A = 14          # wave 1a: rows 0..13  (tiles 0..2)
    R1B = 38          # wave 1b: rows 14..37 (tiles 3..7 + phase B)
    RG1 = 52          # wave 3 group 1: rows 38..51 (tiles 8..11)

    from contextlib import ExitStack
    with ExitStack() as ctx:
        xa_t = ctx.enter_context(nc.sbuf_tensor([128, FLAT], bf))
        x8_t = ctx.enter_context(nc.sbuf_tensor([128, 2 * FLAT8], f8))
        xb_t = ctx.enter_context(nc.sbuf_tensor([128, FLAT], bf))
        xc_t = ctx.enter_context(nc.sbuf_tensor([128, FLAT], bf))
        wt_t = ctx.enter_context(nc.sbuf_tensor([128, 3 * NSLOT * 128], bf))
        wdr_t = ctx.enter_context(nc.sbuf_tensor([128, 3 * 2 * 256], f8))
        bt_t = ctx.enter_context(nc.sbuf_tensor([128, 3], f32))
        vt_t = ctx.enter_context(nc.sbuf_tensor([128, HHALF * W], bf))
        st_t = ctx.enter_context(nc.sbuf_tensor([128, 3 * HHALF * W], bf))
        ps_t = ctx.enter_context(nc.psum_tensor([128, NPS * NFREE], f32))
        d1a = ctx.enter_context(nc.semaphore("d1a"))
        d1b = ctx.enter_context(nc.semaphore("d1b"))
        dbc = ctx.enter_context(nc.semaphore("dbc"))    # wt sBC m0 + xb/xc rows<38
        dvt1 = ctx.enter_context(nc.semaphore("dvt1"))  # bt + vt first half
        dvt2 = ctx.enter_context(nc.semaphore("dvt2"))  # vt second half
        dg1 = ctx.enter_context(nc.semaphore("dg1"))    # x rows 38..51
        dg2 = ctx.enter_context(nc.semaphore("dg2"))    # x rows 52..65
        dw2 = ctx.enter_context(nc.semaphore("dw2"))    # wt/wdr m1,m2
        pes = ctx.enter_context(nc.semaphore("pes"))
        dve = ctx.enter_context(nc.semaphore("dve"))
        dout = ctx.enter_context(nc.semaphore("dout"))
        warm = ctx.enter_context(nc.semaphore("warm"))
        block = ctx.enter_context(nc.Block())
        xa_v = xa_t[:].rearrange("p (r c) -> p r c", c=WP)
        xb_v = xb_t[:].rearrange("p (r c) -> p r c", c=WP)
        xc_v = xc_t[:].rearrange("p (r c) -> p r c", c=WP)
        # [p, two, r, c]: dim 1 is the DoubleRow pair (second copy is the
        # +1-row-shifted image, so slicing rows j0.. gives kh=0 and kh=1)
        x8_v = x8_t[:].rearrange("p (two r c) -> p two r c", two=2, c=WP8)

        # bf16 slots: (weight-slot index, view, kh, kw)
        slots_a = [(s, xa_v, kh, kw) for s, (kh, kw) in enumerate(TAPS_BF)]
        slots_bc = [(SBC0 + i, xb_v, ta[0], ta[1])
                    for i, (ta, _tb) in enumerate(PAIRS_XB)]
        slots_bc += [(SBC0 + 5, xc_v, PAIR_XC[0][0], PAIR_XC[0][1])]

        def wt_ap(m, s):
            o = (m * NSLOT + s) * 128
            return wt_t[:, o:o + 128]

        def wdr_ap(m, pr):
            o = (m * 2 + pr) * 256
            return wdr_t[:, o:o + 256].rearrange("p (two m1) -> p two m1", two=2)

        def emit_phase_a(tensor, k, start):
            """2 DR + 8 bf16 xa matmuls of tile k (no stop)."""
            m, blk = divmod(k, NBLK)
            j0 = blk * RB
            ps = ps_t[:, (k % NPS) * NFREE:(k % NPS + 1) * NFREE]
            for pr, kw in enumerate(DR_KW):
                nc.tensor.matmul(
                    ps, wdr_ap(m, pr),
                    x8_v[:, :, j0: j0 + RB, kw: kw + W],
                    start=(start and pr == 0), stop=False, perf_mode=DR,
                )
            for s, view, kh, kw in slots_a:
                nc.tensor.matmul(
                    ps, wt_ap(m, s),
                    view[:, j0 + kh: j0 + kh + RB, kw: kw + W],
                    start=False, stop=False,
                )

        def emit_phase_b(tensor, k):
            """6 bf16 xb/xc pair matmuls of tile k (stop on last)."""
            m, blk = divmod(k, NBLK)
            j0 = blk * RB
            ps = ps_t[:, (k % NPS) * NFREE:(k % NPS + 1) * NFREE]
            n = len(slots_bc)
            for i, (s, view, kh, kw) in enumerate(slots_bc):
                mm = nc.tensor.matmul(
                    ps, wt_ap(m, s),
                    view[:, j0 + kh: j0 + kh + RB, kw: kw + W],
                    start=False, stop=(i == n - 1),
                )
                if i == n - 1:
                    mm.then_inc(pes, 1)

        @block.sync
        def _(sync):
            # Queues drain FIFO, so pure issue order gives wave priority.
            def splitn(dst, src, lo, hi, sem, n):
                step = ((hi - lo) // n // 16) * 16
                for i in range(n):
                    a = lo + i * step
                    b = hi if i == n - 1 else a + step
                    sync.dma_start(dst[:, a:b], src[:, a:b]).then_inc(sem, 16)

            # wave 1a: m0 weights for phase A + x rows 0..13  (10 DMAs)
            splitn(wt_t, wt_d, 0, SBC0 * 128, d1a, 2)
            sync.dma_start(wdr_t[:, 0:512], wdr_d[:, 0:512]).then_inc(d1a, 16)
            splitn(xa_t, xa_d, 0, R1A * WP, d1a, 3)
            splitn(x8_t, x8_d, 0, R1A * WP8, d1a, 2)
            splitn(x8_t, x8_d, FLAT8, FLAT8 + R1A * WP8, d1a, 2)
            # wave 1b: x rows 14..37 for tiles 3..7  (7 DMAs)
            splitn(xa_t, xa_d, R1A * WP, R1B * WP, d1b, 3)
            splitn(x8_t, x8_d, R1A * WP8, R1B * WP8, d1b, 2)
            splitn(x8_t, x8_d, FLAT8 + R1A * WP8, FLAT8 + R1B * WP8, d1b, 2)
            # wave 2: epilogue inputs for tiles 0..7, then phase-B inputs
            sync.dma_start(bt_t[:], bt_d[:]).then_inc(dvt1, 16)
            splitn(vt_t, vt_d, 0, (HHALF // 2) * W, dvt1, 4)
            sync.dma_start(wt_t[:, SBC0 * 128:NSLOT * 128],
                           wt_d[:, SBC0 * 128:NSLOT * 128]).then_inc(dbc, 16)
            splitn(xb_t, xb_d, 0, R1B * WP, dbc, 3)
            splitn(xc_t, xc_d, 0, R1B * WP, dbc, 3)
            # wave 3 group 1: all x, rows 38..51 (tiles 8..11)
            splitn(xa_t, xa_d, R1B * WP, RG1 * WP, dg1, 2)
            splitn(x8_t, x8_d, R1B * WP8, RG1 * WP8, dg1, 1)
            splitn(x8_t, x8_d, FLAT8 + R1B * WP8, FLAT8 + RG1 * WP8, dg1, 1)
            splitn(xb_t, xb_d, R1B * WP, RG1 * WP, dg1, 2)
            splitn(xc_t, xc_d, R1B * WP, RG1 * WP, dg1, 2)
            # vt second half (DVE needs it from tile 8)
            splitn(vt_t, vt_d, (HHALF // 2) * W, HHALF * W, dvt2, 3)
            # wave 3 group 2: all x, rows 52..65 (tiles 12..15)
            splitn(xa_t, xa_d, RG1 * WP, FLAT, dg2, 2)
            splitn(x8_t, x8_d, RG1 * WP8, FLAT8, dg2, 1)
            splitn(x8_t, x8_d, FLAT8 + RG1 * WP8, 2 * FLAT8, dg2, 1)
            splitn(xb_t, xb_d, RG1 * WP, FLAT, dg2, 2)
            splitn(xc_t, xc_d, RG1 * WP, FLAT, dg2, 2)
            # m1/m2 weights (needed from tile 16)
            splitn(wt_t, wt_d, NSLOT * 128, 3 * NSLOT * 128, dw2, 2)
            sync.dma_start(wdr_t[:, 512:1536], wdr_d[:, 512:1536]).then_inc(dw2, 16)

            # output chunks of OCH tiles; final chunk tapers into quarters
            nch = NT // OCH
            ninc = 0
            for c in range(nch):
                lo, hi = c * OCH * NFREE, (c + 1) * OCH * NFREE
                if c == nch - 1:
                    # tile 46, then tile 47 in four quarter pieces
                    sync.wait_ge(dve, NT - 1)
                    mid = lo + NFREE
                    sync.dma_start(out_d[:, lo:mid], st_t[:, lo:mid]).then_inc(dout, 16)
                    q = NFREE // 4
                    sync.wait_ge(dve, NT)
                    for i in range(2):
                        a = mid + i * q
                        sync.dma_start(out_d[:, a:a + q], st_t[:, a:a + q]).then_inc(dout, 16)
                    sync.wait_ge(dve, NT + 1)
                    for i in range(2, 4):
                        a = mid + i * q
                        sync.dma_start(out_d[:, a:a + q], st_t[:, a:a + q]).then_inc(dout, 16)
                    ninc += 5
                else:
                    sync.wait_ge(dve, OCH * (c + 1))
                    sync.dma_start(out_d[:, lo:hi], st_t[:, lo:hi]).then_inc(dout, 16)
                    ninc += 1
            sync.wait_ge(dout, 16 * ninc)

        @block.tensor
        def _(tensor):
            # pre-warm the PE HAM clock gate during the initial DMA wait:
            # dummy matmuls (zeros into bank 7, which tile 7 later clears
            # with start=True) keep the PE busy so the clock ramps before
            # the real stream begins. st_t is idle SBUF.
            tensor.wait_ge(warm, 1)
            for _ in range(NDUMMY):
                nc.tensor.matmul(
                    ps_t[:, 7 * NFREE:8 * NFREE],
                    st_t[0:1, 0:128],
                    st_t[0:1, 0:NFREE],
                    start=True,
                    stop=True,
                )
            # phase A: DR+xa accumulation for tiles 0..PHA-1, gated on the
            # just-in-time x row chunks
            tensor.wait_ge(d1a, 160)
            for k in range(3):
                emit_phase_a(tensor, k, start=True)
            tensor.wait_ge(d1b, 112)
            for k in range(3, PHA):
                emit_phase_a(tensor, k, start=True)
            # phase B: finish tiles 0..PHA-1 with the xb/xc pair slots
            tensor.wait_ge(dbc, 112)
            for k in range(PHA):
                emit_phase_b(tensor, k)
            # steady state; x rows 38..51 then 52..65 arrive in two waves.
            # one bank-reuse wait covers 4 tiles: tiles k..k+3 need at most
            # dve >= k+3-(NPS-1) = k-4, and DVE lags PE by well under the
            # 3-tile slack this leaves. Fewer waits = fewer PE queue stalls.
            tensor.wait_ge(dg1, 128)
            for k in range(PHA, NT - 1):
                if k == 12:
                    tensor.wait_ge(dg2, 128)
                if k == 16:
                    tensor.wait_ge(dw2, 48)
                if (k - PHA) % 4 == 0:
                    tensor.wait_ge(dve, min(k + 3, NT - 1) - NPS + 1)
                emit_phase_a(tensor, k, start=True)
                emit_phase_b(tensor, k)
            # final tile split into two 2-row groups (N=256 in half banks):
            # the first half's epilogue+DMA overlaps the second half's
            # matmuls, shortening the kernel tail
            k = NT - 1
            m, blk = divmod(k, NBLK)
            j0 = blk * RB
            for h in range(2):
                # halves in DIFFERENT banks (7, then 6): DVE reads half 1
                # while PE accumulates half 2, and same-bank PE-write +
                # DVE-read is a fatal PSUM collision. Bank 6 (tile 46) is
                # free once dve >= NT-1.
                if h == 1:
                    tensor.wait_ge(dve, NT - 1)
                ps_h = ps_t[:, (7 - h) * NFREE:(7 - h) * NFREE + NFREE // 2]
                for pr, kw in enumerate(DR_KW):
                    nc.tensor.matmul(
                        ps_h, wdr_ap(m, pr),
                        x8_v[:, :, j0 + 2 * h: j0 + 2 * h + RB // 2, kw: kw + W],
                        start=(pr == 0), stop=False, perf_mode=DR,
                    )
                for sl, is_last in ((slots_a, False), (slots_bc, True)):
                    n = len(sl)
                    for i, (s, view, kh, kw) in enumerate(sl):
                        mm = nc.tensor.matmul(
                            ps_h, wt_ap(m, s),
                            view[:, j0 + 2 * h + kh: j0 + 2 * h + kh + RB // 2,
                                 kw: kw + W],
                            start=False,
                            stop=(is_last and i == n - 1),
                        )
                        if is_last and i == n - 1:
                            mm.then_inc(pes, 1)

        @block.vector
        def _(vector):
            nc.vector.memset(st_t[0:1, 0:NFREE], 0.0).then_inc(warm, 1)
            vector.wait_ge(dvt1, 80)  # bias + first half of valid resident
            for k in range(NT - 1):
                m, blk = divmod(k, NBLK)
                if k == 8:
                    vector.wait_ge(dvt2, 48)
                ps = ps_t[:, (k % NPS) * NFREE:(k % NPS + 1) * NFREE]
                vector.wait_ge(pes, k + 1)
                nc.vector.scalar_tensor_tensor(
                    st_t[:, k * NFREE:(k + 1) * NFREE],
                    ps,
                    bt_t[:, m:m + 1],
                    vt_t[:, blk * NFREE:(blk + 1) * NFREE],
                    mybir.AluOpType.add,
                    mybir.AluOpType.mult,
                ).then_inc(dve, 1)
            # final tile: two half-width epilogues matching the split groups
            k = NT - 1
            m, blk = divmod(k, NBLK)
            HF = NFREE // 2
            for h in range(2):
                ps_h = ps_t[:, (7 - h) * NFREE:(7 - h) * NFREE + HF]
                vector.wait_ge(pes, k + 1 + h)
                nc.vector.scalar_tensor_tensor(
                    st_t[:, k * NFREE + h * HF:k * NFREE + (h + 1) * HF],
                    ps_h,
                    bt_t[:, m:m + 1],
                    vt_t[:, blk * NFREE + h * HF:blk * NFREE + (h + 1) * HF],
                    mybir.AluOpType.add,
                    mybir.AluOpType.mult,
                ).then_inc(dve, 1)
    return nc


def _causal_mask():
    m = np.ones((KH, KW), dtype=np.float32)
    m[KH // 2, KW // 2:] = 0.0
    m[KH // 2 + 1:, :] = 0.0
    return m


def _prepare_in_maps(x, weight, bias, mask):
    # window-any of mask -> valid [B, H, W] float32
    ind = (np.asarray(mask)[:, 0] != 0)
    indp = np.zeros((B, H + 2 * PAD, W + 2 * PAD), dtype=bool)
    indp[:, PAD:PAD + H, PAD:PAD + W] = ind
    valid = np.zeros((B, H, W), dtype=bool)
    for dh in range(KH):
        for dw in range(KW):
            valid |= indp[:, dh:dh + H, dw:dw + W]
    valid_f = valid.astype(np.float32)

    w32 = np.asarray(weight, dtype=np.float32) * _causal_mask()[None, None]
    w_bf = w32.astype(BF16)

    # bf16 weight slots, m-major: [128 ch-part, m, s, 128 cout]
    wt = np.zeros((3, NSLOT, 128, 128), dtype=BF16)
    for m in range(3):
        cs = slice(m * 128, (m + 1) * 128)
        for s, (kh, kw) in enumerate(TAPS_BF):
            wt[m, s] = w_bf[cs, 0:128, kh, kw].T
        for i, (ta, tb) in enumerate(PAIRS_XB):
            wt[m, SBC0 + i, 0:64] = w_bf[cs, 128:192, ta[0], ta[1]].T
            wt[m, SBC0 + i, 64:128] = w_bf[cs, 128:192, tb[0], tb[1]].T
        ta, tb = PAIR_XC
        wt[m, SBC0 + 5, 0:64] = w_bf[cs, 128:192, ta[0], ta[1]].T
        wt[m, SBC0 + 5, 64:128] = w_bf[cs, 128:192, tb[0], tb[1]].T
    wt_sb = np.ascontiguousarray(wt.transpose(2, 0, 1, 3)).reshape(128, -1)

    # fp8 DR weights: [128 ch, m, pair, two, 128 cout], scaled by 8
    wdr = np.zeros((3, 2, 2, 128, 128), dtype=F8)
    for m in range(3):
        cs = slice(m * 128, (m + 1) * 128)
        for pr, kw in enumerate(DR_KW):
            wdr[m, pr, 0] = (w32[cs, 0:128, 0, kw].T * 8.0).astype(F8)
            wdr[m, pr, 1] = (w32[cs, 0:128, 1, kw].T * 8.0).astype(F8)
    wdr_sb = np.ascontiguousarray(wdr.transpose(3, 0, 1, 2, 4)).reshape(128, -1)

    bias_t = np.ascontiguousarray(
        np.asarray(bias, dtype=np.float32).reshape(3, 128).T)

    x32 = np.asarray(x, dtype=np.float32)
    x_bf = x32.astype(BF16)
    x_f8 = (x32[:, 0:128] / 8.0).astype(F8)   # only ci[0:128] needed in fp8

    in_maps = []
    for c in range(NCORES):
        b, half = c // 2, c % 2
        r0 = half * HHALF
        lo = r0 - PAD
        src_lo = max(lo, 0)
        xp = np.zeros((CIN, NROWS, WP), dtype=BF16)
        xp[:, src_lo - lo:, PAD:PAD + W] = x_bf[b, :, src_lo:r0 + HHALF, :]
        xf = xp.reshape(CIN, FLAT)
        # fp8 staging: same rows, 144-col pitch, plus a +1-row-shifted copy
        xp8 = np.zeros((128, NROWS, WP8), dtype=F8)
        xp8[:, src_lo - lo:, PAD:PAD + W] = x_f8[b, :, src_lo:r0 + HHALF, :]
        x8a = xp8.reshape(128, FLAT8)
        x8b = np.zeros_like(x8a)
        x8b[:, :-WP8] = x8a[:, WP8:]
        x2 = xf[128:192]
        sh1 = np.zeros_like(x2)
        sh1[:, :-1] = x2[:, 1:]
        shr = np.zeros_like(x2)
        shr[:, :-WP] = x2[:, WP:]
        vrow = valid_f[b, r0:r0 + HHALF].reshape(1, HHALF * W).astype(BF16)
        vt = np.ascontiguousarray(np.broadcast_to(vrow, (128, HHALF * W)))
        in_maps.append({
            "xa": np.ascontiguousarray(xf[0:128]),
            "x8": np.ascontiguousarray(np.concatenate([x8a, x8b], axis=1)),
            "xb": np.ascontiguousarray(np.concatenate([x2, sh1], axis=0)),
            "xc": np.ascontiguousarray(np.concatenate([x2, shr], axis=0)),
            "wt": wt_sb,
            "wdr": wdr_sb,
            "bt": bias_t,
            "vt": vt,
        })
    return in_maps


def _assemble(results):
    out_full = np.zeros((B, COUT, H, W), dtype=np.float32)
    for c in range(NCORES):
        b, half = c // 2, c % 2
        o = np.asarray(results[c]["out"]).astype(np.float32)
        o4 = o.reshape(128, 3, HHALF, W).transpose(1, 0, 2, 3).reshape(COUT, HHALF, W)
        out_full[b, :, half * HHALF:(half + 1) * HHALF, :] = o4
    return out_full


def kernel(x, weight, bias, mask, _trace=False):
    in_maps = _prepare_in_maps(x, weight, bias, mask)
    nc = _build_program()
    res = run_bass_kernel_spmd(nc, in_maps, core_ids=list(range(NCORES)),
                               trace=_trace)
    out = _assemble(res.results)
    if _trace:
        return out, res
    return out
